# revision 1
# baseline (speedup 1.0000x reference)
"""GridCrossAttention distributed across 8 NeuronCores.

Sharding (per spec hint): the cell/grid dimension s (and N_q = 16*s) is
sharded across the 8 cores in contiguous blocks; the small kv grid and all
weights are replicated so each core gathers its neighborhoods locally
(nh_idx is unrestricted, so full kv replication subsumes the halo exchange).

kernel(**inputs) takes the FULL unsharded inputs and returns the FULL
output; sharding/unsharding happens inside.
"""

import numpy as np
import jax
import jax.numpy as jnp

# Problem shapes (hardcoded per spec)
B, V, T = 1, 1, 2
C, H = 128, 4
HD = C // H
NKV = 12288
S = NKV
NH = 9
NPQ = 16
NQ = S * NPQ          # 196608
M = 8                 # NeuronCores
S_LOC = S // M        # 1536 cells per core
NQ_LOC = S_LOC * NPQ  # 24576 query rows per core

_ARG_ORDER = [
    "x_q", "x_kv", "nh_idx", "nh_mask", "ln_q_s", "ln_q_b", "Wq", "bq",
    "ln_kv_s", "ln_kv_b", "Wkv", "bkv", "Wa_q", "W_out", "b_out", "gamma",
    "ln_m_s", "ln_m_b", "Wm", "bm", "W1", "b1", "W2", "b2", "gamma_mlp",
]


def _ln(x, scale, bias, eps=1e-5):
    mu = jnp.mean(x, axis=-1, keepdims=True)
    var = jnp.mean(jnp.square(x - mu), axis=-1, keepdims=True)
    return (x - mu) * jax.lax.rsqrt(var + eps) * scale + bias


def _local(x_q, x_kv, nh_idx, nh_mask, ln_q_s, ln_q_b, Wq, bq,
           ln_kv_s, ln_kv_b, Wkv, bkv, Wa_q, W_out, b_out, gamma,
           ln_m_s, ln_m_b, Wm, bm, W1, b1, W2, b2, gamma_mlp):
    """Per-core computation on an S_LOC-cell shard (mirrors the reference)."""
    q = _ln(x_q, ln_q_s, ln_q_b) @ Wq + bq                    # [B,V,T,NQ_LOC,C]
    kv = _ln(x_kv, ln_kv_s, ln_kv_b) @ Wkv + bkv              # [B,V,T,NKV,2C]
    kv_nh = kv[:, :, :, nh_idx, :]                            # [B,V,T,S_LOC,NH,2C]
    k, vv = jnp.split(kv_nh, 2, axis=-1)
    q = q.reshape(B, V, T, S_LOC, NPQ, C)
    qh = (q @ Wa_q).reshape(B, V, T, S_LOC, NPQ, H, HD)
    kh = k.reshape(B, V, T, S_LOC, NH, H, HD)
    vh = vv.reshape(B, V, T, S_LOC, NH, H, HD)
    scores = jnp.einsum('bvtsqhd,bvtsnhd->bvtshqn', qh, kh) / np.sqrt(HD)
    bias = jnp.where(nh_mask, 0.0, -1e9).astype(scores.dtype)
    scores = scores + bias[None, None, None, :, None, None, :]
    attn = jax.nn.softmax(scores, axis=-1)
    out = jnp.einsum('bvtshqn,bvtsnhd->bvtsqhd', attn, vh)
    out = out.reshape(B, V, T, S_LOC, NPQ, C)
    out = out @ W_out + b_out
    x = x_q + gamma * out.reshape(B, V, T, NQ_LOC, C)
    h = _ln(x, ln_m_s, ln_m_b) @ Wm + bm
    m = jax.nn.gelu(h @ W1 + b1) @ W2 + b2
    return x + gamma_mlp * m


_pfwd = jax.pmap(_local)

_cache = {}


def _shard(inputs):
    """Build the per-core argument list (leading axis = 8 cores)."""
    args = []
    for name in _ARG_ORDER:
        a = np.asarray(inputs[name])
        if name == "x_q":
            # contiguous S-blocks: NQ rows split into 8 blocks of 24576
            a = a.reshape(B, V, T, M, NQ_LOC, C)
            a = np.moveaxis(a, 3, 0)                 # [M,B,V,T,NQ_LOC,C]
            a = np.ascontiguousarray(a)
        elif name in ("nh_idx", "nh_mask"):
            a = a.reshape(M, S_LOC, NH)
        else:
            # x_kv and all weights: replicate
            a = np.broadcast_to(a, (M,) + a.shape)
        args.append(a)
    return args


def kernel(**inputs):
    args = _shard(inputs)
    out = _pfwd(*args)                               # [M,B,V,T,NQ_LOC,C]
    out = np.asarray(out)
    out = np.moveaxis(out, 0, 3).reshape(B, V, T, NQ, C)
    return out.astype(np.float32)
